# revision 9
# baseline (speedup 1.0000x reference)
"""Trainium2 Bass kernel for nn_Encoder (bidirectional gated encoder).

Math (per batch element, from the reference):
    xf[0] = u[0];       xf[t] = WB  @ u[t]    for t >= 1
    xb[T-1] = u[T-1];   xb[t] = WBf @ u[t+1]  for t <= T-2
    glogit  = xf @ w1.T + xb @ w2.T + b
    gamma   = sigmoid(glogit)
    y       = gamma * (xf + xb) + (1 - gamma) * u = u + gamma * (s - u),  s = xf + xb
    final   = sigmoid(y @ Wc.T)

Device strategy (SPMD over 8 cores, one batch element per core):
  - s is accumulated directly in PSUM: two matmul groups into the same banks,
    group A = u[t] x [WB.T | a1 | a3 | wc | w1 | w2], group B (token window
    shifted by +1) = u[t+1] x [WBf.T | a2 | a4 | 0 | 0 | 0], where
    a1 = w1@WB, a2 = w2@WBf, a3 = Wc@WB, a4 = Wc@WBf (host-precomputed).
    The 5 extra columns therefore hold glogit-b, s@Wc, u@Wc, w1@u, w2@u.
  - u is transposed on-chip (PE transpose) so d sits on partitions; matmuls
    run as fp32r (full PE rate at N=512).
  - Boundary rows t=0 (group-A column zeroed via a copied lhsT tile) and
    t=T-1 (group B truncated to M=127) are fixed with 1-partition ops.
"""

import numpy as np

import concourse.bacc as bacc
import concourse.mybir as mybir
from concourse import masks, tile
from concourse.bass_utils import run_bass_kernel_spmd

B, T, D = 8, 4096, 1024
P = 128
NCH = D // P            # 8 contraction chunks
NT = T // P             # 32 token tiles
SEG_TILES = 4           # token tiles per segment
NSEG = NT // SEG_TILES
SEG_TOK = SEG_TILES * P
NE = D + 6              # matmul rhs columns (5 used + 1 pad; fp32r needs even N)
F32 = mybir.dt.float32
F32R = mybir.dt.float32r
SIG = mybir.ActivationFunctionType.Sigmoid
MULT = mybir.AluOpType.mult
ADD = mybir.AluOpType.add

_BANKS = ((0, 512), (512, 512), (D, 6))


def _emit_body(nc, tc, pools, tensors):
    cpool, utpool, upool, wpool, spool, pspool, ptpool = pools
    u_d, y_d, g_d, f_d, wa_sb, wb_sb, b_sb, ident, ut0z = tensors

    micro = None
    for seg in range(NSEG):
        t0 = seg * SEG_TOK
        n_utiles = SEG_TILES + 1 if seg < NSEG - 1 else SEG_TILES
        ut = utpool.tile([P, NCH, SEG_TOK + P], F32R, tag="ut")
        u_tiles = []
        for j in range(n_utiles):
            tok = t0 + j * P
            u_sb = upool.tile([P, D], F32, tag="u")
            nc.sync.dma_start(u_sb[:], u_d[tok : tok + P, :])
            u_tiles.append(u_sb)
            for h in range(2):
                pt = ptpool.tile([P, 4, P], F32, tag="pt")
                for cc in range(4):
                    c = 4 * h + cc
                    nc.tensor.transpose(
                        pt[:, cc, :], u_sb[:, c * P : (c + 1) * P], ident[:]
                    )
                nc.vector.tensor_copy(
                    ut[:, 4 * h : 4 * h + 4, j * P : (j + 1) * P], pt[:]
                )

        if seg == 0:
            # lhsT for tile 0 / group A: token-0 column zeroed so row 0 of
            # the psum gets only the group-B (xb) contribution.
            nc.vector.tensor_copy(ut0z[:], ut[:, :, 0:P])
            nc.vector.tensor_scalar_mul(ut0z[:, :, 0:1], ut0z[:, :, 0:1], 0.0)
            # micro[t] = [wc@u[t], w1@u[t]] for tokens 0..127 (row 0 used)
            micro_ps = ptpool.tile([P, 2], F32, tag="pt")
            for c in range(NCH):
                nc.tensor.matmul(
                    micro_ps[:],
                    ut[:, c, 0:P],
                    wa_sb[:, c, D + 2 : D + 4],
                    start=(c == 0),
                    stop=(c == NCH - 1),
                )
            micro = spool.tile([1, 2], F32, tag="mic")
            nc.vector.tensor_copy(micro[:], micro_ps[0:1, :])

        for j in range(SEG_TILES):
            k = seg * SEG_TILES + j
            tok = k * P
            w = j * P
            ps = pspool.tile([P, NE], F32, tag="ps")
            u_sb = u_tiles[j]

            for n0, nn in _BANKS:
                for gi, wsb in ((0, wa_sb), (1, wb_sb)):
                    for c in range(NCH):
                        if gi == 0:
                            lhsT = ut0z[:, c, :] if k == 0 else ut[:, c, w : w + P]
                            out_ap = ps[:, n0 : n0 + nn]
                        elif k == NT - 1:
                            lhsT = ut[:, c, w + 1 : w + P]
                            out_ap = ps[0 : P - 1, n0 : n0 + nn]
                        else:
                            lhsT = ut[:, c, w + 1 : w + P + 1]
                            out_ap = ps[:, n0 : n0 + nn]
                        nc.tensor.matmul(
                            out_ap,
                            lhsT,
                            wsb[:, c, n0 : n0 + nn],
                            start=(gi == 0 and c == 0),
                            stop=(gi == 1 and c == NCH - 1),
                        )

            # stage the 5 extra columns in SBUF, then fix boundary rows there
            ex = spool.tile([P, 5], F32, tag="ex")
            nc.vector.tensor_copy(ex[:], ps[:, D : D + 5])
            if k == 0:
                nc.vector.tensor_add(ex[0:1, 0:1], ex[0:1, 0:1], micro[0:1, 1:2])
                nc.vector.tensor_add(ex[0:1, 1:2], ex[0:1, 1:2], micro[0:1, 0:1])
                nc.vector.tensor_add(ex[0:1, 2:3], ex[0:1, 2:3], micro[0:1, 0:1])
            if k == NT - 1:
                # row-127 only (mask = identity col 127): ex0 += w2@u, ex1 += wc@u
                mask = ident[:, P - 1 : P]
                nc.vector.scalar_tensor_tensor(
                    ex[:, 0:1], ex[:, 4:5], mask, ex[:, 0:1], op0=MULT, op1=ADD
                )
                nc.vector.scalar_tensor_tensor(
                    ex[:, 1:2], ex[:, 2:3], mask, ex[:, 1:2], op0=MULT, op1=ADD
                )

            g_sb = spool.tile([P, 1], F32, tag="g")
            nc.scalar.activation(g_sb[:], ex[:, 0:1], SIG, bias=b_sb[:])

            d_sb = wpool.tile([P, D], F32, tag="d")
            nc.vector.tensor_sub(d_sb[:], ps[:, 0:D], u_sb[:])
            if k == 0:
                nc.vector.tensor_copy(d_sb[0:1, :], ps[0:1, 0:D])
            if k == NT - 1:
                # undo the -u on row 127 only: d += u * mask127
                nc.vector.scalar_tensor_tensor(
                    d_sb[:], u_sb[:], ident[:, P - 1 : P], d_sb[:], op0=MULT, op1=ADD
                )

            y_sb = wpool.tile([P, D], F32, tag="y")
            nc.vector.scalar_tensor_tensor(
                y_sb[:], d_sb[:], g_sb[:], u_sb[:], op0=MULT, op1=ADD
            )

            fl1 = spool.tile([P, 1], F32, tag="fl1")
            nc.vector.tensor_sub(fl1[:], ex[:, 1:2], ex[:, 2:3])
            fl2 = spool.tile([P, 1], F32, tag="fl2")
            nc.vector.scalar_tensor_tensor(
                fl2[:], fl1[:], g_sb[:], ex[:, 2:3], op0=MULT, op1=ADD
            )
            f_sb = spool.tile([P, 1], F32, tag="f")
            nc.scalar.activation(f_sb[:], fl2[:], SIG)

            nc.sync.dma_start(y_d[tok : tok + P, :], y_sb[:])
            nc.sync.dma_start(g_d[tok : tok + P, :], g_sb[:])
            nc.sync.dma_start(f_d[tok : tok + P, :], f_sb[:])


def build_module(reps=1):
    nc = bacc.Bacc("TRN2", target_bir_lowering=False, debug=True)
    u_d = nc.dram_tensor("u", [T, D], F32, kind="ExternalInput")
    wa_d = nc.dram_tensor("wa", [D, NE], F32R, kind="ExternalInput")
    wb_d = nc.dram_tensor("wb", [D, NE], F32R, kind="ExternalInput")
    b_d = nc.dram_tensor("bb", [P, 1], F32, kind="ExternalInput")
    y_d = nc.dram_tensor("y", [T, D], F32, kind="ExternalOutput")
    g_d = nc.dram_tensor("g", [T, 1], F32, kind="ExternalOutput")
    f_d = nc.dram_tensor("f", [T, 1], F32, kind="ExternalOutput")

    with tile.TileContext(nc) as tc:
        with (
            tc.tile_pool(name="const", bufs=1) as cpool,
            tc.tile_pool(name="ut", bufs=1) as utpool,
            tc.tile_pool(name="u", bufs=SEG_TILES + 5) as upool,
            tc.tile_pool(name="work", bufs=3) as wpool,
            tc.tile_pool(name="small", bufs=4) as spool,
            tc.tile_pool(name="psum", bufs=2, space="PSUM") as pspool,
            tc.tile_pool(name="psum_t", bufs=2, space="PSUM") as ptpool,
        ):
            ident = cpool.tile([P, P], F32)
            masks.make_identity(nc, ident[:])
            b_sb = cpool.tile([P, 1], F32)
            nc.sync.dma_start(b_sb[:], b_d[:])
            wa_sb = cpool.tile([P, NCH, NE], F32R)
            wb_sb = cpool.tile([P, NCH, NE], F32R)
            for c in range(NCH):
                nc.sync.dma_start(wa_sb[:, c, :], wa_d[c * P : (c + 1) * P, :])
                nc.sync.dma_start(wb_sb[:, c, :], wb_d[c * P : (c + 1) * P, :])
            ut0z = cpool.tile([P, NCH, P], F32R)

            pools = (cpool, utpool, upool, wpool, spool, pspool, ptpool)
            tensors = (u_d, y_d, g_d, f_d, wa_sb, wb_sb, b_sb, ident, ut0z)
            for _ in range(reps):
                _emit_body(nc, tc, pools, tensors)

    nc.finalize()
    return nc


def prep_inputs(u, WA, WB, WAf, WBf, w1, w2, b, Wc):
    u = np.ascontiguousarray(np.asarray(u, np.float32))
    WB64 = np.asarray(WB, np.float64)
    WBf64 = np.asarray(WBf, np.float64)
    w1_ = np.asarray(w1, np.float64)[0]
    w2_ = np.asarray(w2, np.float64)[0]
    wc_ = np.asarray(Wc, np.float64)[0]

    wa_cat = np.zeros((D, NE), np.float32)
    wa_cat[:, :D] = np.asarray(WB, np.float32).T
    wa_cat[:, D] = (w1_ @ WB64).astype(np.float32)
    wa_cat[:, D + 1] = (wc_ @ WB64).astype(np.float32)
    wa_cat[:, D + 2] = wc_.astype(np.float32)
    wa_cat[:, D + 3] = w1_.astype(np.float32)
    wa_cat[:, D + 4] = w2_.astype(np.float32)

    wb_cat = np.zeros((D, NE), np.float32)
    wb_cat[:, :D] = np.asarray(WBf, np.float32).T
    wb_cat[:, D] = (w2_ @ WBf64).astype(np.float32)
    wb_cat[:, D + 1] = (wc_ @ WBf64).astype(np.float32)

    b128 = np.ascontiguousarray(
        np.broadcast_to(np.asarray(b, np.float32).reshape(1, 1), (P, 1))
    )
    in_maps = [
        {
            "u": np.ascontiguousarray(u[i]),
            "wa": wa_cat,
            "wb": wb_cat,
            "bb": b128,
        }
        for i in range(B)
    ]
    return in_maps


_NC_CACHE = {}


def get_module(reps=1):
    if reps not in _NC_CACHE:
        _NC_CACHE[reps] = build_module(reps)
    return _NC_CACHE[reps]


def run_device(in_maps, reps=1):
    nc = get_module(reps)
    res = run_bass_kernel_spmd(nc, in_maps, core_ids=list(range(B)))
    return res


def kernel(u, WA, WB, WAf, WBf, w1, w2, b, Wc):
    in_maps = prep_inputs(u, WA, WB, WAf, WBf, w1, w2, b, Wc)
    res = run_device(in_maps)
    y = np.stack([res.results[i]["y"] for i in range(B)])
    gamma = np.stack([res.results[i]["g"] for i in range(B)])
    final = np.stack([res.results[i]["f"] for i in range(B)]).reshape(B, T)
    return y, gamma, final


# revision 12
# speedup vs baseline: 1.0377x; 1.0377x over previous
"""Trainium2 Bass kernel for nn_Encoder (bidirectional gated encoder).

Math (per batch element, from the reference):
    xf[0] = u[0];       xf[t] = WB  @ u[t]    for t >= 1
    xb[T-1] = u[T-1];   xb[t] = WBf @ u[t+1]  for t <= T-2
    glogit  = xf @ w1.T + xb @ w2.T + b
    gamma   = sigmoid(glogit)
    y       = gamma * (xf + xb) + (1 - gamma) * u = u + gamma * (s - u),  s = xf + xb
    final   = sigmoid(y @ Wc.T)

Device strategy (SPMD over 8 cores, one batch element per core):
  - s is accumulated directly in PSUM: two matmul groups into the same banks,
    group A = u[t] x [WB.T | a1 | a3 | wc | w1 | w2], group B (token window
    shifted by +1) = u[t+1] x [WBf.T | a2 | a4 | 0 | 0 | 0], where
    a1 = w1@WB, a2 = w2@WBf, a3 = Wc@WB, a4 = Wc@WBf (host-precomputed).
    The 5 extra columns therefore hold glogit-b, s@Wc, u@Wc, w1@u, w2@u.
  - u is transposed on-chip (PE transpose) so d sits on partitions; matmuls
    run as fp32r (full PE rate at N=512).
  - Boundary rows t=0 (group-A column zeroed via a copied lhsT tile) and
    t=T-1 (group B truncated to M=127) are fixed with 1-partition ops.
"""

import numpy as np

import concourse.bacc as bacc
import concourse.mybir as mybir
from concourse import masks, tile
from concourse.bass_utils import run_bass_kernel_spmd

B, T, D = 8, 4096, 1024
P = 128
NCH = D // P            # 8 contraction chunks
NT = T // P             # 32 token tiles
SEG_TILES = 4           # token tiles per segment
NSEG = NT // SEG_TILES
SEG_TOK = SEG_TILES * P
NE = D + 6              # matmul rhs columns (5 used + 1 pad; fp32r needs even N)
F32 = mybir.dt.float32
F32R = mybir.dt.float32r
SIG = mybir.ActivationFunctionType.Sigmoid
MULT = mybir.AluOpType.mult
ADD = mybir.AluOpType.add

_BANKS = ((0, 512), (512, 512), (D, 6))


def _emit_body(nc, tc, pools, tensors):
    cpool, utpool, upool, wpool, spool, pspool, ptpool = pools
    u_d, y_d, g_d, f_d, wa_sb, wb_sb, b_sb, ident, ident_r, ut0z = tensors

    micro = None
    for seg in range(NSEG):
        t0 = seg * SEG_TOK
        n_utiles = SEG_TILES + 1 if seg < NSEG - 1 else SEG_TILES
        ut = utpool.tile([P, NCH, SEG_TOK + P], F32R, tag="ut")
        u_tiles = []
        for j in range(n_utiles):
            tok = t0 + j * P
            u_sb = upool.tile([P, D], F32R, tag="u")
            nc.sync.dma_start(u_sb[:], u_d[tok : tok + P, :])
            u_tiles.append(u_sb)
            for h in range(2):
                pt = ptpool.tile([P, 4, P], F32R, tag="pt")
                for cc in range(4):
                    c = 4 * h + cc
                    nc.tensor.transpose(
                        pt[:, cc, :], u_sb[:, c * P : (c + 1) * P], ident_r[:]
                    )
                nc.vector.tensor_copy(
                    ut[:, 4 * h : 4 * h + 4, j * P : (j + 1) * P], pt[:]
                )

        if seg == 0:
            # lhsT for tile 0 / group A: token-0 column zeroed so row 0 of
            # the psum gets only the group-B (xb) contribution.
            nc.vector.tensor_copy(ut0z[:], ut[:, :, 0:P])
            nc.vector.tensor_scalar_mul(ut0z[:, :, 0:1], ut0z[:, :, 0:1], 0.0)
            # micro[t] = [wc@u[t], w1@u[t]] for tokens 0..127 (row 0 used)
            micro_ps = ptpool.tile([P, 2], F32, tag="pt")
            for c in range(NCH):
                nc.tensor.matmul(
                    micro_ps[:],
                    ut[:, c, 0:P],
                    wa_sb[:, c, D + 2 : D + 4],
                    start=(c == 0),
                    stop=(c == NCH - 1),
                )
            micro = spool.tile([1, 2], F32, tag="mic")
            nc.vector.tensor_copy(micro[:], micro_ps[0:1, :])

        for j in range(SEG_TILES):
            k = seg * SEG_TILES + j
            tok = k * P
            w = j * P
            ps = pspool.tile([P, NE], F32, tag="ps")
            u_sb = u_tiles[j]

            # lhsT-outer ordering: the three bank matmuls sharing one
            # stationary operand are consecutive, minimizing weight reloads.
            for gi, wsb in ((0, wa_sb), (1, wb_sb)):
                for c in range(NCH):
                    if gi == 0:
                        lhsT = ut0z[:, c, :] if k == 0 else ut[:, c, w : w + P]
                        rows = P
                    elif k == NT - 1:
                        lhsT = ut[:, c, w + 1 : w + P]
                        rows = P - 1
                    else:
                        lhsT = ut[:, c, w + 1 : w + P + 1]
                        rows = P
                    for n0, nn in _BANKS:
                        nc.tensor.matmul(
                            ps[0:rows, n0 : n0 + nn],
                            lhsT,
                            wsb[:, c, n0 : n0 + nn],
                            start=(gi == 0 and c == 0),
                            stop=(gi == 1 and c == NCH - 1),
                            skip_group_check=True,
                        )

            # stage the 5 extra columns in SBUF, then fix boundary rows there
            ex = spool.tile([P, 5], F32, tag="ex")
            nc.vector.tensor_copy(ex[:], ps[:, D : D + 5])
            if k == 0:
                nc.vector.tensor_add(ex[0:1, 0:1], ex[0:1, 0:1], micro[0:1, 1:2])
                nc.vector.tensor_add(ex[0:1, 1:2], ex[0:1, 1:2], micro[0:1, 0:1])
                nc.vector.tensor_add(ex[0:1, 2:3], ex[0:1, 2:3], micro[0:1, 0:1])
            if k == NT - 1:
                # row-127 only (mask = identity col 127): ex0 += w2@u, ex1 += wc@u
                mask = ident[:, P - 1 : P]
                nc.vector.scalar_tensor_tensor(
                    ex[:, 0:1], ex[:, 4:5], mask, ex[:, 0:1], op0=MULT, op1=ADD
                )
                nc.vector.scalar_tensor_tensor(
                    ex[:, 1:2], ex[:, 2:3], mask, ex[:, 1:2], op0=MULT, op1=ADD
                )

            g_sb = spool.tile([P, 1], F32, tag="g")
            nc.scalar.activation(g_sb[:], ex[:, 0:1], SIG, bias=b_sb[:])

            d_sb = wpool.tile([P, D], F32, tag="d")
            nc.vector.tensor_sub(d_sb[:], ps[:, 0:D], u_sb[:].bitcast(F32))
            if k == 0:
                nc.vector.tensor_copy(d_sb[0:1, :], ps[0:1, 0:D])
            if k == NT - 1:
                # undo the -u on row 127 only: d += u * mask127
                nc.vector.scalar_tensor_tensor(
                    d_sb[:], u_sb[:].bitcast(F32), ident[:, P - 1 : P], d_sb[:], op0=MULT, op1=ADD
                )

            y_sb = wpool.tile([P, D], F32, tag="y")
            nc.vector.scalar_tensor_tensor(
                y_sb[:], d_sb[:], g_sb[:], u_sb[:].bitcast(F32), op0=MULT, op1=ADD
            )

            fl1 = spool.tile([P, 1], F32, tag="fl1")
            nc.vector.tensor_sub(fl1[:], ex[:, 1:2], ex[:, 2:3])
            fl2 = spool.tile([P, 1], F32, tag="fl2")
            nc.vector.scalar_tensor_tensor(
                fl2[:], fl1[:], g_sb[:], ex[:, 2:3], op0=MULT, op1=ADD
            )
            f_sb = spool.tile([P, 1], F32, tag="f")
            nc.scalar.activation(f_sb[:], fl2[:], SIG)

            nc.sync.dma_start(y_d[tok : tok + P, :], y_sb[:])
            nc.sync.dma_start(g_d[tok : tok + P, :], g_sb[:])
            nc.sync.dma_start(f_d[tok : tok + P, :], f_sb[:])


def build_module(reps=1):
    nc = bacc.Bacc("TRN2", target_bir_lowering=False, debug=True)
    u_d = nc.dram_tensor("u", [T, D], F32R, kind="ExternalInput")
    wa_d = nc.dram_tensor("wa", [D, NE], F32R, kind="ExternalInput")
    wb_d = nc.dram_tensor("wb", [D, NE], F32R, kind="ExternalInput")
    b_d = nc.dram_tensor("bb", [P, 1], F32, kind="ExternalInput")
    y_d = nc.dram_tensor("y", [T, D], F32, kind="ExternalOutput")
    g_d = nc.dram_tensor("g", [T, 1], F32, kind="ExternalOutput")
    f_d = nc.dram_tensor("f", [T, 1], F32, kind="ExternalOutput")

    with tile.TileContext(nc) as tc:
        with (
            tc.tile_pool(name="const", bufs=1) as cpool,
            tc.tile_pool(name="ut", bufs=1) as utpool,
            tc.tile_pool(name="u", bufs=SEG_TILES + 5) as upool,
            tc.tile_pool(name="work", bufs=3) as wpool,
            tc.tile_pool(name="small", bufs=4) as spool,
            tc.tile_pool(name="psum", bufs=2, space="PSUM") as pspool,
            tc.tile_pool(name="psum_t", bufs=2, space="PSUM") as ptpool,
        ):
            ident = cpool.tile([P, P], F32)
            masks.make_identity(nc, ident[:])
            ident_r = cpool.tile([P, P], F32R)
            nc.vector.tensor_copy(ident_r[:], ident[:])
            b_sb = cpool.tile([P, 1], F32)
            nc.sync.dma_start(b_sb[:], b_d[:])
            wa_sb = cpool.tile([P, NCH, NE], F32R)
            wb_sb = cpool.tile([P, NCH, NE], F32R)
            for c in range(NCH):
                nc.sync.dma_start(wa_sb[:, c, :], wa_d[c * P : (c + 1) * P, :])
                nc.sync.dma_start(wb_sb[:, c, :], wb_d[c * P : (c + 1) * P, :])
            ut0z = cpool.tile([P, NCH, P], F32R)

            pools = (cpool, utpool, upool, wpool, spool, pspool, ptpool)
            tensors = (u_d, y_d, g_d, f_d, wa_sb, wb_sb, b_sb, ident, ident_r, ut0z)
            for _ in range(reps):
                _emit_body(nc, tc, pools, tensors)

    nc.finalize()
    return nc


def prep_inputs(u, WA, WB, WAf, WBf, w1, w2, b, Wc):
    u = np.ascontiguousarray(np.asarray(u, np.float32))
    WB64 = np.asarray(WB, np.float64)
    WBf64 = np.asarray(WBf, np.float64)
    w1_ = np.asarray(w1, np.float64)[0]
    w2_ = np.asarray(w2, np.float64)[0]
    wc_ = np.asarray(Wc, np.float64)[0]

    wa_cat = np.zeros((D, NE), np.float32)
    wa_cat[:, :D] = np.asarray(WB, np.float32).T
    wa_cat[:, D] = (w1_ @ WB64).astype(np.float32)
    wa_cat[:, D + 1] = (wc_ @ WB64).astype(np.float32)
    wa_cat[:, D + 2] = wc_.astype(np.float32)
    wa_cat[:, D + 3] = w1_.astype(np.float32)
    wa_cat[:, D + 4] = w2_.astype(np.float32)

    wb_cat = np.zeros((D, NE), np.float32)
    wb_cat[:, :D] = np.asarray(WBf, np.float32).T
    wb_cat[:, D] = (w2_ @ WBf64).astype(np.float32)
    wb_cat[:, D + 1] = (wc_ @ WBf64).astype(np.float32)

    b128 = np.ascontiguousarray(
        np.broadcast_to(np.asarray(b, np.float32).reshape(1, 1), (P, 1))
    )
    in_maps = [
        {
            "u": np.ascontiguousarray(u[i]),
            "wa": wa_cat,
            "wb": wb_cat,
            "bb": b128,
        }
        for i in range(B)
    ]
    return in_maps


_NC_CACHE = {}


def get_module(reps=1):
    if reps not in _NC_CACHE:
        _NC_CACHE[reps] = build_module(reps)
    return _NC_CACHE[reps]


def run_device(in_maps, reps=1):
    nc = get_module(reps)
    res = run_bass_kernel_spmd(nc, in_maps, core_ids=list(range(B)))
    return res


def kernel(u, WA, WB, WAf, WBf, w1, w2, b, Wc):
    in_maps = prep_inputs(u, WA, WB, WAf, WBf, w1, w2, b, Wc)
    res = run_device(in_maps)
    y = np.stack([res.results[i]["y"] for i in range(B)])
    gamma = np.stack([res.results[i]["g"] for i in range(B)])
    final = np.stack([res.results[i]["f"] for i in range(B)]).reshape(B, T)
    return y, gamma, final


# revision 21
# speedup vs baseline: 29985.1729x; 28895.9729x over previous
"""Trainium2 Bass kernel for nn_Encoder (bidirectional gated encoder).

Math (per batch element, from the reference):
    xf[0] = u[0];       xf[t] = WB  @ u[t]    for t >= 1
    xb[T-1] = u[T-1];   xb[t] = WBf @ u[t+1]  for t <= T-2
    glogit  = xf @ w1.T + xb @ w2.T + b
    gamma   = sigmoid(glogit)
    y       = gamma * (xf + xb) + (1 - gamma) * u = u + gamma * (s - u),  s = xf + xb
    final   = sigmoid(y @ Wc.T)

Device strategy (SPMD over 8 cores, one batch element per core):
  - s is accumulated directly in PSUM: two matmul groups into the same banks,
    group A = u[t] x [WB.T | a1 | a3 | wc | w1 | w2], group B (token window
    shifted by +1) = u[t+1] x [WBf.T | a2 | a4 | 0 | 0 | 0], where
    a1 = w1@WB, a2 = w2@WBf, a3 = Wc@WB, a4 = Wc@WBf (host-precomputed).
    The 5 extra columns therefore hold glogit-b, s@Wc, u@Wc, w1@u, w2@u.
  - u is transposed on-chip (PE transpose) so d sits on partitions; matmuls
    run as fp32r (full PE rate at N=512).
  - Boundary rows t=0 (group-A column zeroed via a copied lhsT tile) and
    t=T-1 (group B truncated to M=127) are fixed with 1-partition ops.
"""

import numpy as np

import concourse.bacc as bacc
import concourse.mybir as mybir
from concourse import masks, tile
from concourse.bass_utils import run_bass_kernel_spmd

B, T, D = 8, 4096, 1024
P = 128
NCH = D // P            # 8 contraction chunks
NT = T // P             # 32 token tiles
SEG_TILES = 4           # token tiles per segment
NSEG = NT // SEG_TILES
SEG_TOK = SEG_TILES * P
NE = D + 6              # matmul rhs columns (5 used + 1 pad; fp32r needs even N)
F32 = mybir.dt.float32
F32R = mybir.dt.float32r
SIG = mybir.ActivationFunctionType.Sigmoid
MULT = mybir.AluOpType.mult
ADD = mybir.AluOpType.add

_BANKS = ((0, 512), (512, 512), (D, 6))


def _emit_body(nc, tc, pools, tensors):
    cpool, utpool, upool, wpool, spool, pspool, ptpool = pools
    u_d, y_d, g_d, f_d, wa_sb, wb_sb, b_sb, ident, ident_r, ut0z = tensors

    def load_transpose(ut, j, tok):
        """DMA u[tok:tok+P] and PE-transpose it into ut token-window j."""
        u_sb = upool.tile([P, D], F32R, tag="u")
        nc.sync.dma_start(u_sb[:], u_d[tok : tok + P, :])
        for h in range(2):
            pt = ptpool.tile([P, 4, P], F32R, tag="pt")
            for cc in range(4):
                c = 4 * h + cc
                nc.tensor.transpose(
                    pt[:, cc, :], u_sb[:, c * P : (c + 1) * P], ident_r[:]
                )
            nc.vector.tensor_copy(
                ut[:, 4 * h : 4 * h + 4, j * P : (j + 1) * P], pt[:]
            )
        return u_sb

    def n_utiles(seg):
        return SEG_TILES + 1 if seg < NSEG - 1 else SEG_TILES

    micro = None
    ut_bufs = {}
    u_tiles_by_seg = {}
    for seg in range(NSEG):
        t0 = seg * SEG_TOK
        if seg == 0:
            # prologue: fill segment 0's transposed buffer upfront
            ut = utpool.tile([P, NCH, SEG_TOK + P], F32R, tag="ut")
            ut_bufs[0] = ut
            u_tiles_by_seg[0] = [
                load_transpose(ut, j, t0 + j * P) for j in range(n_utiles(0))
            ]
        ut = ut_bufs.pop(seg)
        u_tiles = u_tiles_by_seg.pop(seg)

        if seg == 0:
            # lhsT for tile 0 / group A: token-0 column zeroed so row 0 of
            # the psum gets only the group-B (xb) contribution.
            nc.vector.tensor_copy(ut0z[:], ut[:, :, 0:P])
            nc.vector.tensor_scalar_mul(ut0z[:, :, 0:1], ut0z[:, :, 0:1], 0.0)
            # micro[t] = [wc@u[t], w1@u[t]] for tokens 0..127 (row 0 used)
            micro_ps = ptpool.tile([P, 2], F32, tag="pt")
            for c in range(NCH):
                nc.tensor.matmul(
                    micro_ps[:],
                    ut[:, c, 0:P],
                    wa_sb[:, c, D + 2 : D + 4],
                    start=(c == 0),
                    stop=(c == NCH - 1),
                )
            micro = spool.tile([1, 2], F32, tag="mic")
            nc.vector.tensor_copy(micro[:], micro_ps[0:1, :])

        for j in range(SEG_TILES):
            k = seg * SEG_TILES + j
            tok = k * P
            w = j * P
            ps = pspool.tile([P, NE], F32, tag="ps")
            u_sb = u_tiles[j]

            # lhsT-outer ordering: the three bank matmuls sharing one
            # stationary operand are consecutive, minimizing weight reloads.
            for gi, wsb in ((0, wa_sb), (1, wb_sb)):
                for c in range(NCH):
                    if gi == 0:
                        lhsT = ut0z[:, c, :] if k == 0 else ut[:, c, w : w + P]
                        rows = P
                    elif k == NT - 1:
                        lhsT = ut[:, c, w + 1 : w + P]
                        rows = P - 1
                    else:
                        lhsT = ut[:, c, w + 1 : w + P + 1]
                        rows = P
                    for n0, nn in _BANKS:
                        nc.tensor.matmul(
                            ps[0:rows, n0 : n0 + nn],
                            lhsT,
                            wsb[:, c, n0 : n0 + nn],
                            start=(gi == 0 and c == 0),
                            stop=(gi == 1 and c == NCH - 1),
                            skip_group_check=True,
                        )

            # stage the 5 extra columns in SBUF, then fix boundary rows there
            ex = spool.tile([P, 5], F32, tag="ex")
            nc.vector.tensor_copy(ex[:], ps[:, D : D + 5])
            if k == 0:
                nc.vector.tensor_add(ex[0:1, 0:1], ex[0:1, 0:1], micro[0:1, 1:2])
                nc.vector.tensor_add(ex[0:1, 1:2], ex[0:1, 1:2], micro[0:1, 0:1])
                nc.vector.tensor_add(ex[0:1, 2:3], ex[0:1, 2:3], micro[0:1, 0:1])
            if k == NT - 1:
                # row-127 only (mask = identity col 127): ex0 += w2@u, ex1 += wc@u
                mask = ident[:, P - 1 : P]
                nc.vector.scalar_tensor_tensor(
                    ex[:, 0:1], ex[:, 4:5], mask, ex[:, 0:1], op0=MULT, op1=ADD
                )
                nc.vector.scalar_tensor_tensor(
                    ex[:, 1:2], ex[:, 2:3], mask, ex[:, 1:2], op0=MULT, op1=ADD
                )

            g_sb = spool.tile([P, 1], F32, tag="g")
            nc.scalar.activation(g_sb[:], ex[:, 0:1], SIG, bias=b_sb[:])

            # y = gamma*ps + cu*u, cu = 1-gamma (interior rows; ps == s there).
            # Boundary rows have ps = s - u, so cu must be 1.0 there instead.
            t1 = wpool.tile([P, D], F32, tag="d")
            nc.scalar.activation(
                t1[:], ps[:, 0:D], mybir.ActivationFunctionType.Identity, scale=g_sb[:]
            )
            cu = spool.tile([P, 1], F32, tag="cu")
            nc.vector.tensor_scalar(cu[:], g_sb[:], -1.0, 1.0, op0=MULT, op1=ADD)
            if k == 0:
                nc.vector.scalar_tensor_tensor(
                    cu[:], g_sb[:], ident[:, 0:1], cu[:], op0=MULT, op1=ADD
                )
            if k == NT - 1:
                nc.vector.scalar_tensor_tensor(
                    cu[:], g_sb[:], ident[:, P - 1 : P], cu[:], op0=MULT, op1=ADD
                )

            y_sb = wpool.tile([P, D], F32, tag="y")
            nc.vector.scalar_tensor_tensor(
                y_sb[:], u_sb[:].bitcast(F32), cu[:], t1[:], op0=MULT, op1=ADD
            )

            fl1 = spool.tile([P, 1], F32, tag="fl1")
            nc.vector.tensor_sub(fl1[:], ex[:, 1:2], ex[:, 2:3])
            fl2 = spool.tile([P, 1], F32, tag="fl2")
            nc.vector.scalar_tensor_tensor(
                fl2[:], fl1[:], g_sb[:], ex[:, 2:3], op0=MULT, op1=ADD
            )
            f_sb = spool.tile([P, 1], F32, tag="f")
            nc.scalar.activation(f_sb[:], fl2[:], SIG)

            nc.sync.dma_start(y_d[tok : tok + P, :], y_sb[:])
            nc.sync.dma_start(g_d[tok : tok + P, :], g_sb[:])
            nc.sync.dma_start(f_d[tok : tok + P, :], f_sb[:])

            # software pipeline: transpose next segment's u-tiles between this
            # segment's matmul tiles so PE never sees a long transpose-only block
            if seg + 1 < NSEG:
                nseg = seg + 1
                nt0 = nseg * SEG_TOK
                if nseg not in ut_bufs:
                    ut_bufs[nseg] = utpool.tile(
                        [P, NCH, SEG_TOK + P], F32R, tag="ut", name=f"ut{nseg}"
                    )
                    u_tiles_by_seg[nseg] = []
                todo = [j] if j < SEG_TILES - 1 else list(range(j, n_utiles(nseg)))
                for jj in todo:
                    u_tiles_by_seg[nseg].append(
                        load_transpose(ut_bufs[nseg], jj, nt0 + jj * P)
                    )


def build_module(reps=1):
    nc = bacc.Bacc("TRN2", target_bir_lowering=False, debug=True)
    u_d = nc.dram_tensor("u", [T, D], F32R, kind="ExternalInput")
    wa_d = nc.dram_tensor("wa", [D, NE], F32R, kind="ExternalInput")
    wb_d = nc.dram_tensor("wb", [D, NE], F32R, kind="ExternalInput")
    b_d = nc.dram_tensor("bb", [P, 1], F32, kind="ExternalInput")
    y_d = nc.dram_tensor("y", [T, D], F32, kind="ExternalOutput")
    g_d = nc.dram_tensor("g", [T, 1], F32, kind="ExternalOutput")
    f_d = nc.dram_tensor("f", [T, 1], F32, kind="ExternalOutput")

    with tile.TileContext(nc) as tc:
        with (
            tc.tile_pool(name="const", bufs=1) as cpool,
            tc.tile_pool(name="ut", bufs=2) as utpool,
            tc.tile_pool(name="u", bufs=2 * SEG_TILES + 3) as upool,
            tc.tile_pool(name="work", bufs=3) as wpool,
            tc.tile_pool(name="small", bufs=4) as spool,
            tc.tile_pool(name="psum", bufs=2, space="PSUM") as pspool,
            tc.tile_pool(name="psum_t", bufs=2, space="PSUM") as ptpool,
        ):
            ident = cpool.tile([P, P], F32)
            masks.make_identity(nc, ident[:])
            ident_r = cpool.tile([P, P], F32R)
            nc.vector.tensor_copy(ident_r[:], ident[:])
            b_sb = cpool.tile([P, 1], F32)
            nc.sync.dma_start(b_sb[:], b_d[:])
            wa_sb = cpool.tile([P, NCH, NE], F32R)
            wb_sb = cpool.tile([P, NCH, NE], F32R)
            for c in range(NCH):
                nc.sync.dma_start(wa_sb[:, c, :], wa_d[c * P : (c + 1) * P, :])
                nc.sync.dma_start(wb_sb[:, c, :], wb_d[c * P : (c + 1) * P, :])
            ut0z = cpool.tile([P, NCH, P], F32R)

            pools = (cpool, utpool, upool, wpool, spool, pspool, ptpool)
            tensors = (u_d, y_d, g_d, f_d, wa_sb, wb_sb, b_sb, ident, ident_r, ut0z)
            for _ in range(reps):
                _emit_body(nc, tc, pools, tensors)

    nc.finalize()
    return nc


def prep_inputs(u, WA, WB, WAf, WBf, w1, w2, b, Wc):
    u = np.ascontiguousarray(np.asarray(u, np.float32))
    WB64 = np.asarray(WB, np.float64)
    WBf64 = np.asarray(WBf, np.float64)
    w1_ = np.asarray(w1, np.float64)[0]
    w2_ = np.asarray(w2, np.float64)[0]
    wc_ = np.asarray(Wc, np.float64)[0]

    wa_cat = np.zeros((D, NE), np.float32)
    wa_cat[:, :D] = np.asarray(WB, np.float32).T
    wa_cat[:, D] = (w1_ @ WB64).astype(np.float32)
    wa_cat[:, D + 1] = (wc_ @ WB64).astype(np.float32)
    wa_cat[:, D + 2] = wc_.astype(np.float32)
    wa_cat[:, D + 3] = w1_.astype(np.float32)
    wa_cat[:, D + 4] = w2_.astype(np.float32)

    wb_cat = np.zeros((D, NE), np.float32)
    wb_cat[:, :D] = np.asarray(WBf, np.float32).T
    wb_cat[:, D] = (w2_ @ WBf64).astype(np.float32)
    wb_cat[:, D + 1] = (wc_ @ WBf64).astype(np.float32)

    b128 = np.ascontiguousarray(
        np.broadcast_to(np.asarray(b, np.float32).reshape(1, 1), (P, 1))
    )
    in_maps = [
        {
            "u": np.ascontiguousarray(u[i]),
            "wa": wa_cat,
            "wb": wb_cat,
            "bb": b128,
        }
        for i in range(B)
    ]
    return in_maps


_NC_CACHE = {}


def get_module(reps=1):
    if reps not in _NC_CACHE:
        _NC_CACHE[reps] = build_module(reps)
    return _NC_CACHE[reps]


def run_device(in_maps, reps=1):
    nc = get_module(reps)
    res = run_bass_kernel_spmd(nc, in_maps, core_ids=list(range(B)))
    return res


def kernel(u, WA, WB, WAf, WBf, w1, w2, b, Wc):
    in_maps = prep_inputs(u, WA, WB, WAf, WBf, w1, w2, b, Wc)
    res = run_device(in_maps)
    y = np.stack([res.results[i]["y"] for i in range(B)])
    gamma = np.stack([res.results[i]["g"] for i in range(B)])
    final = np.stack([res.results[i]["f"] for i in range(B)]).reshape(B, T)
    return y, gamma, final


# revision 22
# speedup vs baseline: 30446.5937x; 1.0154x over previous
"""Trainium2 Bass kernel for nn_Encoder (bidirectional gated encoder).

Math (per batch element, from the reference):
    xf[0] = u[0];       xf[t] = WB  @ u[t]    for t >= 1
    xb[T-1] = u[T-1];   xb[t] = WBf @ u[t+1]  for t <= T-2
    glogit  = xf @ w1.T + xb @ w2.T + b
    gamma   = sigmoid(glogit)
    y       = gamma * (xf + xb) + (1 - gamma) * u = u + gamma * (s - u),  s = xf + xb
    final   = sigmoid(y @ Wc.T)

Device strategy (SPMD over 8 cores, one batch element per core):
  - s is accumulated directly in PSUM: two matmul groups into the same banks,
    group A = u[t] x [WB.T | a1 | a3 | wc | w1 | w2], group B (token window
    shifted by +1) = u[t+1] x [WBf.T | a2 | a4 | 0 | 0 | 0], where
    a1 = w1@WB, a2 = w2@WBf, a3 = Wc@WB, a4 = Wc@WBf (host-precomputed).
    The 5 extra columns therefore hold glogit-b, s@Wc, u@Wc, w1@u, w2@u.
  - u is transposed on-chip (PE transpose) so d sits on partitions; matmuls
    run as fp32r (full PE rate at N=512).
  - Boundary rows t=0 (group-A column zeroed via a copied lhsT tile) and
    t=T-1 (group B truncated to M=127) are fixed with 1-partition ops.
"""

import numpy as np

import concourse.bacc as bacc
import concourse.mybir as mybir
from concourse import masks, tile
from concourse.bass_utils import run_bass_kernel_spmd

B, T, D = 8, 4096, 1024
P = 128
NCH = D // P            # 8 contraction chunks
NT = T // P             # 32 token tiles
SEG_TILES = 4           # token tiles per segment
NSEG = NT // SEG_TILES
SEG_TOK = SEG_TILES * P
NE = D + 6              # matmul rhs columns (5 used + 1 pad; fp32r needs even N)
F32 = mybir.dt.float32
F32R = mybir.dt.float32r
SIG = mybir.ActivationFunctionType.Sigmoid
MULT = mybir.AluOpType.mult
ADD = mybir.AluOpType.add

_BANKS = ((0, 512), (512, 512), (D, 6))


def _emit_body(nc, tc, pools, tensors):
    cpool, utpool, upool, wpool, spool, pspool, ptpool = pools
    u_d, y_d, g_d, f_d, wa_sb, wb_sb, b_sb, ident, ident_r, ut0z = tensors

    def load_transpose(ut, j, tok):
        """DMA u[tok:tok+P] and PE-transpose it into ut token-window j."""
        u_sb = upool.tile([P, D], F32R, tag="u")
        nc.sync.dma_start(u_sb[:], u_d[tok : tok + P, :])
        for h in range(2):
            pt = ptpool.tile([P, 4, P], F32R, tag="pt")
            for cc in range(4):
                c = 4 * h + cc
                nc.tensor.transpose(
                    pt[:, cc, :], u_sb[:, c * P : (c + 1) * P], ident_r[:]
                )
            nc.scalar.activation(
                ut[:, 4 * h : 4 * h + 4, j * P : (j + 1) * P],
                pt[:],
                mybir.ActivationFunctionType.Copy,
            )
        return u_sb

    def n_utiles(seg):
        return SEG_TILES + 1 if seg < NSEG - 1 else SEG_TILES

    micro = None
    ut_bufs = {}
    u_tiles_by_seg = {}
    for seg in range(NSEG):
        t0 = seg * SEG_TOK
        if seg == 0:
            # prologue: fill segment 0's transposed buffer upfront
            ut = utpool.tile([P, NCH, SEG_TOK + P], F32R, tag="ut")
            ut_bufs[0] = ut
            u_tiles_by_seg[0] = [
                load_transpose(ut, j, t0 + j * P) for j in range(n_utiles(0))
            ]
        ut = ut_bufs.pop(seg)
        u_tiles = u_tiles_by_seg.pop(seg)

        if seg == 0:
            # lhsT for tile 0 / group A: token-0 column zeroed so row 0 of
            # the psum gets only the group-B (xb) contribution.
            nc.vector.tensor_copy(ut0z[:], ut[:, :, 0:P])
            nc.vector.tensor_scalar_mul(ut0z[:, :, 0:1], ut0z[:, :, 0:1], 0.0)
            # micro[t] = [wc@u[t], w1@u[t]] for tokens 0..127 (row 0 used)
            micro_ps = ptpool.tile([P, 2], F32, tag="pt")
            for c in range(NCH):
                nc.tensor.matmul(
                    micro_ps[:],
                    ut[:, c, 0:P],
                    wa_sb[:, c, D + 2 : D + 4],
                    start=(c == 0),
                    stop=(c == NCH - 1),
                )
            micro = spool.tile([1, 2], F32, tag="mic")
            nc.vector.tensor_copy(micro[:], micro_ps[0:1, :])

        for j in range(SEG_TILES):
            k = seg * SEG_TILES + j
            tok = k * P
            w = j * P
            ps = pspool.tile([P, NE], F32, tag="ps")
            u_sb = u_tiles[j]

            # lhsT-outer ordering: the three bank matmuls sharing one
            # stationary operand are consecutive, minimizing weight reloads.
            for gi, wsb in ((0, wa_sb), (1, wb_sb)):
                for c in range(NCH):
                    if gi == 0:
                        lhsT = ut0z[:, c, :] if k == 0 else ut[:, c, w : w + P]
                        rows = P
                    elif k == NT - 1:
                        lhsT = ut[:, c, w + 1 : w + P]
                        rows = P - 1
                    else:
                        lhsT = ut[:, c, w + 1 : w + P + 1]
                        rows = P
                    for n0, nn in _BANKS:
                        nc.tensor.matmul(
                            ps[0:rows, n0 : n0 + nn],
                            lhsT,
                            wsb[:, c, n0 : n0 + nn],
                            start=(gi == 0 and c == 0),
                            stop=(gi == 1 and c == NCH - 1),
                            skip_group_check=True,
                        )

            # stage the 5 extra columns in SBUF, then fix boundary rows there
            ex = spool.tile([P, 5], F32, tag="ex")
            nc.vector.tensor_copy(ex[:], ps[:, D : D + 5])
            if k == 0:
                nc.vector.tensor_add(ex[0:1, 0:1], ex[0:1, 0:1], micro[0:1, 1:2])
                nc.vector.tensor_add(ex[0:1, 1:2], ex[0:1, 1:2], micro[0:1, 0:1])
                nc.vector.tensor_add(ex[0:1, 2:3], ex[0:1, 2:3], micro[0:1, 0:1])
            if k == NT - 1:
                # row-127 only (mask = identity col 127): ex0 += w2@u, ex1 += wc@u
                mask = ident[:, P - 1 : P]
                nc.vector.scalar_tensor_tensor(
                    ex[:, 0:1], ex[:, 4:5], mask, ex[:, 0:1], op0=MULT, op1=ADD
                )
                nc.vector.scalar_tensor_tensor(
                    ex[:, 1:2], ex[:, 2:3], mask, ex[:, 1:2], op0=MULT, op1=ADD
                )

            g_sb = spool.tile([P, 1], F32, tag="g")
            nc.scalar.activation(g_sb[:], ex[:, 0:1], SIG, bias=b_sb[:])

            # y = gamma*ps + cu*u, cu = 1-gamma (interior rows; ps == s there).
            # Boundary rows have ps = s - u, so cu must be 1.0 there instead.
            t1 = wpool.tile([P, D], F32, tag="d")
            nc.scalar.activation(
                t1[:], ps[:, 0:D], mybir.ActivationFunctionType.Identity, scale=g_sb[:]
            )
            cu = spool.tile([P, 1], F32, tag="cu")
            nc.vector.tensor_scalar(cu[:], g_sb[:], -1.0, 1.0, op0=MULT, op1=ADD)
            if k == 0:
                nc.vector.scalar_tensor_tensor(
                    cu[:], g_sb[:], ident[:, 0:1], cu[:], op0=MULT, op1=ADD
                )
            if k == NT - 1:
                nc.vector.scalar_tensor_tensor(
                    cu[:], g_sb[:], ident[:, P - 1 : P], cu[:], op0=MULT, op1=ADD
                )

            y_sb = wpool.tile([P, D], F32, tag="y")
            nc.vector.scalar_tensor_tensor(
                y_sb[:], u_sb[:].bitcast(F32), cu[:], t1[:], op0=MULT, op1=ADD
            )

            fl1 = spool.tile([P, 1], F32, tag="fl1")
            nc.vector.tensor_sub(fl1[:], ex[:, 1:2], ex[:, 2:3])
            fl2 = spool.tile([P, 1], F32, tag="fl2")
            nc.vector.scalar_tensor_tensor(
                fl2[:], fl1[:], g_sb[:], ex[:, 2:3], op0=MULT, op1=ADD
            )
            f_sb = spool.tile([P, 1], F32, tag="f")
            nc.scalar.activation(f_sb[:], fl2[:], SIG)

            nc.sync.dma_start(y_d[tok : tok + P, :], y_sb[:])
            nc.sync.dma_start(g_d[tok : tok + P, :], g_sb[:])
            nc.sync.dma_start(f_d[tok : tok + P, :], f_sb[:])

            # software pipeline: transpose next segment's u-tiles between this
            # segment's matmul tiles so PE never sees a long transpose-only block
            if seg + 1 < NSEG:
                nseg = seg + 1
                nt0 = nseg * SEG_TOK
                if nseg not in ut_bufs:
                    ut_bufs[nseg] = utpool.tile(
                        [P, NCH, SEG_TOK + P], F32R, tag="ut", name=f"ut{nseg}"
                    )
                    u_tiles_by_seg[nseg] = []
                todo = [j] if j < SEG_TILES - 1 else list(range(j, n_utiles(nseg)))
                for jj in todo:
                    u_tiles_by_seg[nseg].append(
                        load_transpose(ut_bufs[nseg], jj, nt0 + jj * P)
                    )


def build_module(reps=1):
    nc = bacc.Bacc("TRN2", target_bir_lowering=False, debug=True)
    u_d = nc.dram_tensor("u", [T, D], F32R, kind="ExternalInput")
    wa_d = nc.dram_tensor("wa", [D, NE], F32R, kind="ExternalInput")
    wb_d = nc.dram_tensor("wb", [D, NE], F32R, kind="ExternalInput")
    b_d = nc.dram_tensor("bb", [P, 1], F32, kind="ExternalInput")
    y_d = nc.dram_tensor("y", [T, D], F32, kind="ExternalOutput")
    g_d = nc.dram_tensor("g", [T, 1], F32, kind="ExternalOutput")
    f_d = nc.dram_tensor("f", [T, 1], F32, kind="ExternalOutput")

    with tile.TileContext(nc) as tc:
        with (
            tc.tile_pool(name="const", bufs=1) as cpool,
            tc.tile_pool(name="ut", bufs=2) as utpool,
            tc.tile_pool(name="u", bufs=2 * SEG_TILES + 3) as upool,
            tc.tile_pool(name="work", bufs=3) as wpool,
            tc.tile_pool(name="small", bufs=4) as spool,
            tc.tile_pool(name="psum", bufs=2, space="PSUM") as pspool,
            tc.tile_pool(name="psum_t", bufs=2, space="PSUM") as ptpool,
        ):
            ident = cpool.tile([P, P], F32)
            masks.make_identity(nc, ident[:])
            ident_r = cpool.tile([P, P], F32R)
            nc.vector.tensor_copy(ident_r[:], ident[:])
            b_sb = cpool.tile([P, 1], F32)
            nc.sync.dma_start(b_sb[:], b_d[:])
            wa_sb = cpool.tile([P, NCH, NE], F32R)
            wb_sb = cpool.tile([P, NCH, NE], F32R)
            for c in range(NCH):
                nc.sync.dma_start(wa_sb[:, c, :], wa_d[c * P : (c + 1) * P, :])
                nc.sync.dma_start(wb_sb[:, c, :], wb_d[c * P : (c + 1) * P, :])
            ut0z = cpool.tile([P, NCH, P], F32R)

            pools = (cpool, utpool, upool, wpool, spool, pspool, ptpool)
            tensors = (u_d, y_d, g_d, f_d, wa_sb, wb_sb, b_sb, ident, ident_r, ut0z)
            for _ in range(reps):
                _emit_body(nc, tc, pools, tensors)

    nc.finalize()
    return nc


def prep_inputs(u, WA, WB, WAf, WBf, w1, w2, b, Wc):
    u = np.ascontiguousarray(np.asarray(u, np.float32))
    WB64 = np.asarray(WB, np.float64)
    WBf64 = np.asarray(WBf, np.float64)
    w1_ = np.asarray(w1, np.float64)[0]
    w2_ = np.asarray(w2, np.float64)[0]
    wc_ = np.asarray(Wc, np.float64)[0]

    wa_cat = np.zeros((D, NE), np.float32)
    wa_cat[:, :D] = np.asarray(WB, np.float32).T
    wa_cat[:, D] = (w1_ @ WB64).astype(np.float32)
    wa_cat[:, D + 1] = (wc_ @ WB64).astype(np.float32)
    wa_cat[:, D + 2] = wc_.astype(np.float32)
    wa_cat[:, D + 3] = w1_.astype(np.float32)
    wa_cat[:, D + 4] = w2_.astype(np.float32)

    wb_cat = np.zeros((D, NE), np.float32)
    wb_cat[:, :D] = np.asarray(WBf, np.float32).T
    wb_cat[:, D] = (w2_ @ WBf64).astype(np.float32)
    wb_cat[:, D + 1] = (wc_ @ WBf64).astype(np.float32)

    b128 = np.ascontiguousarray(
        np.broadcast_to(np.asarray(b, np.float32).reshape(1, 1), (P, 1))
    )
    in_maps = [
        {
            "u": np.ascontiguousarray(u[i]),
            "wa": wa_cat,
            "wb": wb_cat,
            "bb": b128,
        }
        for i in range(B)
    ]
    return in_maps


_NC_CACHE = {}


def get_module(reps=1):
    if reps not in _NC_CACHE:
        _NC_CACHE[reps] = build_module(reps)
    return _NC_CACHE[reps]


def run_device(in_maps, reps=1):
    nc = get_module(reps)
    res = run_bass_kernel_spmd(nc, in_maps, core_ids=list(range(B)))
    return res


def kernel(u, WA, WB, WAf, WBf, w1, w2, b, Wc):
    in_maps = prep_inputs(u, WA, WB, WAf, WBf, w1, w2, b, Wc)
    res = run_device(in_maps)
    y = np.stack([res.results[i]["y"] for i in range(B)])
    gamma = np.stack([res.results[i]["g"] for i in range(B)])
    final = np.stack([res.results[i]["f"] for i in range(B)]).reshape(B, T)
    return y, gamma, final


# revision 23
# speedup vs baseline: 30698.0102x; 1.0083x over previous
"""Trainium2 Bass kernel for nn_Encoder (bidirectional gated encoder).

Math (per batch element, from the reference):
    xf[0] = u[0];       xf[t] = WB  @ u[t]    for t >= 1
    xb[T-1] = u[T-1];   xb[t] = WBf @ u[t+1]  for t <= T-2
    glogit  = xf @ w1.T + xb @ w2.T + b
    gamma   = sigmoid(glogit)
    y       = gamma * (xf + xb) + (1 - gamma) * u = u + gamma * (s - u),  s = xf + xb
    final   = sigmoid(y @ Wc.T)

Device strategy (SPMD over 8 cores, one batch element per core):
  - s is accumulated directly in PSUM: two matmul groups into the same banks,
    group A = u[t] x [WB.T | a1 | a3 | wc | w1 | w2], group B (token window
    shifted by +1) = u[t+1] x [WBf.T | a2 | a4 | 0 | 0 | 0], where
    a1 = w1@WB, a2 = w2@WBf, a3 = Wc@WB, a4 = Wc@WBf (host-precomputed).
    The 5 extra columns therefore hold glogit-b, s@Wc, u@Wc, w1@u, w2@u.
  - u is transposed on-chip (PE transpose) so d sits on partitions; matmuls
    run as fp32r (full PE rate at N=512, ~1e-4 rel err).
  - Next segment's transposes are interleaved between this segment's matmul
    tiles (keeps PE dense / HAM warm); epilogue leans on ScalarE
    (gamma*ps via activation-Identity with a [P,1] scale, psum->sbuf
    evictions) because DVE ops pay a pipeline drain ~= their own duration.
  - Boundary rows t=0 (group-A column zeroed via a copied lhsT tile) and
    t=T-1 (group B truncated to M=127) are fixed via the cu coefficient
    (y = gamma*ps + cu*u with cu=1 on those rows) and identity-column masks.
"""

import numpy as np

import concourse.bacc as bacc
import concourse.mybir as mybir
from concourse import masks, tile
from concourse.bass_utils import run_bass_kernel_spmd

B, T, D = 8, 4096, 1024
P = 128
NCH = D // P            # 8 contraction chunks
NT = T // P             # 32 token tiles
SEG_TILES = 4           # token tiles per segment
NSEG = NT // SEG_TILES
SEG_TOK = SEG_TILES * P
NE = D + 6              # matmul rhs columns (5 used + 1 pad; fp32r needs even N)
F32 = mybir.dt.float32
F32R = mybir.dt.float32r
SIG = mybir.ActivationFunctionType.Sigmoid
MULT = mybir.AluOpType.mult
ADD = mybir.AluOpType.add

_BANKS = ((0, 512), (512, 512), (D, 6))


def _emit_body(nc, tc, pools, tensors):
    cpool, utpool, upool, wpool, spool, pspool, ptpool = pools
    u_d, y_d, g_d, f_d, wa_sb, wb_sb, b_sb, ident, ident_r, ut0z = tensors

    def load_transpose(ut, j, tok):
        """DMA u[tok:tok+P] and PE-transpose it into ut token-window j."""
        u_sb = upool.tile([P, D], F32R, tag="u")
        nc.sync.dma_start(u_sb[:], u_d[tok : tok + P, :])
        for h in range(2):
            pt = ptpool.tile([P, 4, P], F32R, tag="pt")
            for cc in range(4):
                c = 4 * h + cc
                nc.tensor.transpose(
                    pt[:, cc, :], u_sb[:, c * P : (c + 1) * P], ident_r[:]
                )
            nc.scalar.activation(
                ut[:, 4 * h : 4 * h + 4, j * P : (j + 1) * P],
                pt[:],
                mybir.ActivationFunctionType.Copy,
            )
        return u_sb

    def n_utiles(seg):
        return SEG_TILES + 1 if seg < NSEG - 1 else SEG_TILES

    micro = None
    ut_bufs = {}
    u_tiles_by_seg = {}
    for seg in range(NSEG):
        t0 = seg * SEG_TOK
        if seg == 0:
            # prologue: fill segment 0's transposed buffer upfront
            ut = utpool.tile([P, NCH, SEG_TOK + P], F32R, tag="ut")
            ut_bufs[0] = ut
            u_tiles_by_seg[0] = [
                load_transpose(ut, j, t0 + j * P) for j in range(n_utiles(0))
            ]
        ut = ut_bufs.pop(seg)
        u_tiles = u_tiles_by_seg.pop(seg)

        if seg == 0:
            # lhsT for tile 0 / group A: token-0 column zeroed so row 0 of
            # the psum gets only the group-B (xb) contribution.
            nc.vector.tensor_copy(ut0z[:], ut[:, :, 0:P])
            nc.vector.tensor_scalar_mul(ut0z[:, :, 0:1], ut0z[:, :, 0:1], 0.0)
            # micro[t] = [wc@u[t], w1@u[t]] for tokens 0..127 (row 0 used)
            micro_ps = ptpool.tile([P, 2], F32, tag="pt")
            for c in range(NCH):
                nc.tensor.matmul(
                    micro_ps[:],
                    ut[:, c, 0:P],
                    wa_sb[:, c, D + 2 : D + 4],
                    start=(c == 0),
                    stop=(c == NCH - 1),
                )
            micro = spool.tile([1, 2], F32, tag="mic")
            nc.vector.tensor_copy(micro[:], micro_ps[0:1, :])

        for j in range(SEG_TILES):
            k = seg * SEG_TILES + j
            tok = k * P
            w = j * P
            ps = pspool.tile([P, NE], F32, tag="ps")
            u_sb = u_tiles[j]

            # lhsT-outer ordering: the three bank matmuls sharing one
            # stationary operand are consecutive, minimizing weight reloads.
            for gi, wsb in ((0, wa_sb), (1, wb_sb)):
                for c in range(NCH):
                    if gi == 0:
                        lhsT = ut0z[:, c, :] if k == 0 else ut[:, c, w : w + P]
                        rows = P
                    elif k == NT - 1:
                        lhsT = ut[:, c, w + 1 : w + P]
                        rows = P - 1
                    else:
                        lhsT = ut[:, c, w + 1 : w + P + 1]
                        rows = P
                    for n0, nn in _BANKS:
                        nc.tensor.matmul(
                            ps[0:rows, n0 : n0 + nn],
                            lhsT,
                            wsb[:, c, n0 : n0 + nn],
                            start=(gi == 0 and c == 0),
                            stop=(gi == 1 and c == NCH - 1),
                            skip_group_check=True,
                        )

            # stage the 5 extra columns in SBUF, then fix boundary rows there
            ex = spool.tile([P, 5], F32, tag="ex")
            nc.vector.tensor_copy(ex[:], ps[:, D : D + 5])
            if k == 0:
                nc.vector.tensor_add(ex[0:1, 0:1], ex[0:1, 0:1], micro[0:1, 1:2])
                nc.vector.tensor_add(ex[0:1, 1:2], ex[0:1, 1:2], micro[0:1, 0:1])
                nc.vector.tensor_add(ex[0:1, 2:3], ex[0:1, 2:3], micro[0:1, 0:1])
            if k == NT - 1:
                # row-127 only (mask = identity col 127): ex0 += w2@u, ex1 += wc@u
                mask = ident[:, P - 1 : P]
                nc.vector.scalar_tensor_tensor(
                    ex[:, 0:1], ex[:, 4:5], mask, ex[:, 0:1], op0=MULT, op1=ADD
                )
                nc.vector.scalar_tensor_tensor(
                    ex[:, 1:2], ex[:, 2:3], mask, ex[:, 1:2], op0=MULT, op1=ADD
                )

            g_sb = spool.tile([P, 1], F32, tag="g")
            nc.scalar.activation(g_sb[:], ex[:, 0:1], SIG, bias=b_sb[:])

            # y = gamma*ps + cu*u, cu = 1-gamma (interior rows; ps == s there).
            # Boundary rows have ps = s - u, so cu must be 1.0 there instead.
            t1 = wpool.tile([P, D], F32, tag="d")
            nc.scalar.activation(
                t1[:], ps[:, 0:D], mybir.ActivationFunctionType.Identity, scale=g_sb[:]
            )
            cu = spool.tile([P, 1], F32, tag="cu")
            nc.vector.tensor_scalar(cu[:], g_sb[:], -1.0, 1.0, op0=MULT, op1=ADD)
            if k == 0:
                nc.vector.scalar_tensor_tensor(
                    cu[:], g_sb[:], ident[:, 0:1], cu[:], op0=MULT, op1=ADD
                )
            if k == NT - 1:
                nc.vector.scalar_tensor_tensor(
                    cu[:], g_sb[:], ident[:, P - 1 : P], cu[:], op0=MULT, op1=ADD
                )

            y_sb = wpool.tile([P, D], F32, tag="y")
            nc.vector.scalar_tensor_tensor(
                y_sb[:], u_sb[:].bitcast(F32), cu[:], t1[:], op0=MULT, op1=ADD
            )

            fl1 = spool.tile([P, 1], F32, tag="fl1")
            nc.vector.tensor_sub(fl1[:], ex[:, 1:2], ex[:, 2:3])
            fl2 = spool.tile([P, 1], F32, tag="fl2")
            nc.vector.scalar_tensor_tensor(
                fl2[:], fl1[:], g_sb[:], ex[:, 2:3], op0=MULT, op1=ADD
            )
            f_sb = spool.tile([P, 1], F32, tag="f")
            nc.scalar.activation(f_sb[:], fl2[:], SIG)

            nc.sync.dma_start(y_d[tok : tok + P, :], y_sb[:])
            nc.sync.dma_start(g_d[tok : tok + P, :], g_sb[:])
            nc.sync.dma_start(f_d[tok : tok + P, :], f_sb[:])

            # software pipeline: transpose next segment's u-tiles between this
            # segment's matmul tiles so PE never sees a long transpose-only block
            if seg + 1 < NSEG:
                nseg = seg + 1
                nt0 = nseg * SEG_TOK
                if nseg not in ut_bufs:
                    ut_bufs[nseg] = utpool.tile(
                        [P, NCH, SEG_TOK + P], F32R, tag="ut", name=f"ut{nseg}"
                    )
                    u_tiles_by_seg[nseg] = []
                todo = [j] if j < SEG_TILES - 1 else list(range(j, n_utiles(nseg)))
                for jj in todo:
                    u_tiles_by_seg[nseg].append(
                        load_transpose(ut_bufs[nseg], jj, nt0 + jj * P)
                    )


def build_module(reps=1):
    nc = bacc.Bacc("TRN2", target_bir_lowering=False, debug=True)
    u_d = nc.dram_tensor("u", [T, D], F32R, kind="ExternalInput")
    wa_d = nc.dram_tensor("wa", [D, NE], F32R, kind="ExternalInput")
    wb_d = nc.dram_tensor("wb", [D, NE], F32R, kind="ExternalInput")
    b_d = nc.dram_tensor("bb", [P, 1], F32, kind="ExternalInput")
    y_d = nc.dram_tensor("y", [T, D], F32, kind="ExternalOutput")
    g_d = nc.dram_tensor("g", [T, 1], F32, kind="ExternalOutput")
    f_d = nc.dram_tensor("f", [T, 1], F32, kind="ExternalOutput")

    with tile.TileContext(nc) as tc:
        with (
            tc.tile_pool(name="const", bufs=1) as cpool,
            tc.tile_pool(name="ut", bufs=2) as utpool,
            tc.tile_pool(name="u", bufs=2 * SEG_TILES + 3) as upool,
            tc.tile_pool(name="work", bufs=3) as wpool,
            tc.tile_pool(name="small", bufs=4) as spool,
            tc.tile_pool(name="psum", bufs=2, space="PSUM") as pspool,
            tc.tile_pool(name="psum_t", bufs=2, space="PSUM") as ptpool,
        ):
            ident = cpool.tile([P, P], F32)
            masks.make_identity(nc, ident[:])
            ident_r = cpool.tile([P, P], F32R)
            nc.vector.tensor_copy(ident_r[:], ident[:])
            b_sb = cpool.tile([P, 1], F32)
            nc.sync.dma_start(b_sb[:], b_d[:])
            wa_sb = cpool.tile([P, NCH, NE], F32R)
            wb_sb = cpool.tile([P, NCH, NE], F32R)
            for c in range(NCH):
                nc.sync.dma_start(wa_sb[:, c, :], wa_d[c * P : (c + 1) * P, :])
                nc.sync.dma_start(wb_sb[:, c, :], wb_d[c * P : (c + 1) * P, :])
            ut0z = cpool.tile([P, NCH, P], F32R)

            pools = (cpool, utpool, upool, wpool, spool, pspool, ptpool)
            tensors = (u_d, y_d, g_d, f_d, wa_sb, wb_sb, b_sb, ident, ident_r, ut0z)
            for _ in range(reps):
                _emit_body(nc, tc, pools, tensors)

    nc.finalize()
    return nc


def prep_inputs(u, WA, WB, WAf, WBf, w1, w2, b, Wc):
    u = np.ascontiguousarray(np.asarray(u, np.float32))
    WB64 = np.asarray(WB, np.float64)
    WBf64 = np.asarray(WBf, np.float64)
    w1_ = np.asarray(w1, np.float64)[0]
    w2_ = np.asarray(w2, np.float64)[0]
    wc_ = np.asarray(Wc, np.float64)[0]

    wa_cat = np.zeros((D, NE), np.float32)
    wa_cat[:, :D] = np.asarray(WB, np.float32).T
    wa_cat[:, D] = (w1_ @ WB64).astype(np.float32)
    wa_cat[:, D + 1] = (wc_ @ WB64).astype(np.float32)
    wa_cat[:, D + 2] = wc_.astype(np.float32)
    wa_cat[:, D + 3] = w1_.astype(np.float32)
    wa_cat[:, D + 4] = w2_.astype(np.float32)

    wb_cat = np.zeros((D, NE), np.float32)
    wb_cat[:, :D] = np.asarray(WBf, np.float32).T
    wb_cat[:, D] = (w2_ @ WBf64).astype(np.float32)
    wb_cat[:, D + 1] = (wc_ @ WBf64).astype(np.float32)

    b128 = np.ascontiguousarray(
        np.broadcast_to(np.asarray(b, np.float32).reshape(1, 1), (P, 1))
    )
    in_maps = [
        {
            "u": np.ascontiguousarray(u[i]),
            "wa": wa_cat,
            "wb": wb_cat,
            "bb": b128,
        }
        for i in range(B)
    ]
    return in_maps


_NC_CACHE = {}


def get_module(reps=1):
    if reps not in _NC_CACHE:
        _NC_CACHE[reps] = build_module(reps)
    return _NC_CACHE[reps]


def run_device(in_maps, reps=1):
    nc = get_module(reps)
    res = run_bass_kernel_spmd(nc, in_maps, core_ids=list(range(B)))
    return res


def kernel(u, WA, WB, WAf, WBf, w1, w2, b, Wc):
    in_maps = prep_inputs(u, WA, WB, WAf, WBf, w1, w2, b, Wc)
    res = run_device(in_maps)
    y = np.stack([res.results[i]["y"] for i in range(B)])
    gamma = np.stack([res.results[i]["g"] for i in range(B)])
    final = np.stack([res.results[i]["f"] for i in range(B)]).reshape(B, T)
    return y, gamma, final
